# revision 1
# baseline (speedup 1.0000x reference)
"""Trainium2 Bass kernel for nn_ClawMatrix (cross-modal claw-matrix alignment).

reference computation per batch element b:
    vp = relu(LN(v @ vW + vb) * vg + vbeta)          [S, D]
    lp = relu(LN(l @ lW + lb) * lg + lbeta)          [S, D]
    sim = vp @ lp.T * (mean(claw) / 0.07)            [S, S]
    A   = softmax(sim, axis=-1)
    av  = A @ vp                                     [S, D]
    al  = A.T @ lp                                   [S, D]
    out = relu(LN([av, al] @ oW + ob) * og + obeta)  [S, D]

Sharding: data-parallel over batch B=8 across the 8 NeuronCores (one batch
element per core, weights replicated, no collectives).

Device-side strategy (per core):
  - all matmuls in bf16 (fp32 PSUM accumulation)
  - LN+ReLU fused into one ScalarE activation (scale=rstd, bias=-mean*rstd)
  - exp fused with row-sum via activation accum_out; softmax normalizer 1/D
    folded into lp (for A^T lp) and into a post-scale of av^T (for A vp),
    so the unnormalized E=exp(c*sim) matrix is used directly in both bmms
  - transposed layouts (vpT, lpT, E^T blocks) produced by DMA XBAR transpose
  - combined^T assembled directly in [2D, S] layout so the output projection
    needs no additional transposes
  - SBUF pressure handled with explicitly managed (non-LIFO) pool lifetimes
"""

import os
import sys
import math
import numpy as np

for _p in ("/opt/trn_rl_repo", "/root/.axon_site/_ro/trn_rl_repo"):
    if os.path.isdir(_p) and _p not in sys.path:
        sys.path.insert(0, _p)

import ml_dtypes  # noqa: E402

BF16 = ml_dtypes.bfloat16

P = 128           # partitions
B = 8             # batch / cores
S = 2048          # sequence
D = 768           # feature dim
EPS = 1e-5
TEMPERATURE = 0.07

_BUILD_CACHE = {}


class _Pool:
    """Manually managed tile-pool lifetime (enter now, exit at any point)."""

    def __init__(self, tc, **kw):
        self._cm = tc.tile_pool(**kw)
        self.pool = self._cm.__enter__()
        self._open = True

    def tile(self, *a, **kw):
        if "name" not in kw:
            kw["name"] = kw.get("tag") or f"t{id(self) % 9973}"
        return self.pool.tile(*a, **kw)

    def close(self):
        if self._open:
            self._cm.__exit__(None, None, None)
            self._open = False


def _build(c_scale: float, trivial: bool, s: int = S, d: int = D):
    """Builds the single-core Bass program. Returns the compiled Bacc module."""
    import concourse.bass as bass
    import concourse.tile as tile
    from concourse import bacc, mybir

    f32 = mybir.dt.float32
    bf16 = mybir.dt.bfloat16
    AF = mybir.ActivationFunctionType
    AX = mybir.AxisListType
    OP = mybir.AluOpType

    st_n = s // P          # number of 128-row tiles over S
    dt_n = d // P          # number of 128-row tiles over D
    kt_n = 2 * dt_n        # k tiles over 2D for the output projection
    ch = 512               # matmul free-dim chunk (one PSUM bank of fp32)
    tc_n = s // ch         # chunks over S
    g_sz = ch // P         # s-tiles per 512-column group
    bn_sub = math.gcd(512, d)
    bn_k = d // bn_sub
    d_chunks = [(i, min(ch, d - i)) for i in range(0, d, ch)]

    nc = bacc.Bacc(
        "TRN2",
        target_bir_lowering=False,
        debug=False,
        enable_asserts=False,
        num_devices=B,
    )

    vlT_d = nc.dram_tensor("vlT", [2 * d, s], bf16, kind="ExternalInput")
    vW_d = nc.dram_tensor("vW", [d, d], bf16, kind="ExternalInput")
    lW_d = nc.dram_tensor("lW", [d, d], bf16, kind="ExternalInput")
    oW_d = nc.dram_tensor("oW", [2 * d, d], bf16, kind="ExternalInput")
    if not trivial:
        vb_d = nc.dram_tensor("vb", [1, d], bf16, kind="ExternalInput")
        lb_d = nc.dram_tensor("lb", [1, d], bf16, kind="ExternalInput")
        ob_d = nc.dram_tensor("ob", [1, d], bf16, kind="ExternalInput")
        vg_d = nc.dram_tensor("vg", [1, d], f32, kind="ExternalInput")
        vbe_d = nc.dram_tensor("vbeta", [1, d], f32, kind="ExternalInput")
        lg_d = nc.dram_tensor("lg", [1, d], f32, kind="ExternalInput")
        lbe_d = nc.dram_tensor("lbeta", [1, d], f32, kind="ExternalInput")
        og_d = nc.dram_tensor("og", [1, d], f32, kind="ExternalInput")
        obe_d = nc.dram_tensor("obeta", [1, d], f32, kind="ExternalInput")
    out_d = nc.dram_tensor("out", [s, d], f32, kind="ExternalOutput")
    rscr_d = nc.dram_tensor("rinv_scratch", [st_n, P], f32, kind="Internal")

    with tile.TileContext(nc) as tc:
        pp = _Pool(tc, name="persist", bufs=1)
        sp = _Pool(tc, name="small", bufs=4)

        eps_sb = pp.tile([P, 1], f32, tag='eps')
        nc.vector.memset(eps_sb[:], EPS)
        racc = pp.tile([P, st_n, tc_n], f32, tag='racc')
        rinv_all = pp.tile([P, st_n], f32, tag='rinv_all')

        if not trivial:
            ones_sb = pp.tile([1, P], bf16, tag="ones_sb")
            nc.vector.memset(ones_sb[:], 1.0)
            b_sb = {}
            aff = {}
            for nm, dd in (("vb", vb_d), ("lb", lb_d), ("ob", ob_d)):
                t = pp.tile([1, d], bf16, tag=nm)
                nc.sync.dma_start(out=t[:], in_=dd.ap())
                b_sb[nm] = t
            for nm, dd in (("vg", vg_d), ("vbeta", vbe_d), ("lg", lg_d),
                           ("lbeta", lbe_d), ("og", og_d), ("obeta", obe_d)):
                t = pp.tile([P, d], f32, tag=nm)
                src = bass.AP(tensor=dd.ap().tensor, offset=0,
                              ap=[[0, P], [1, d]])
                nc.sync.dma_start(out=t[:], in_=src)
                aff[nm] = t

        # Address reuse via same-tag slot cycling (bufs=1):
        #   slab64: vT -> lT -> E_all          (64 KB/part)
        #   slab48: (vpT | lpT) -> combT       (48 KB/part)
        slab64 = _Pool(tc, name="slab64", bufs=1)
        slab48 = _Pool(tc, name="slab48", bufs=1)
        vplp_p = _Pool(tc, name="vplp", bufs=1)
        w_p = _Pool(tc, name="wslab", bufs=1)
        at_p = _Pool(tc, name="at", bufs=2)
        rb_p = _Pool(tc, name="rb", bufs=2)
        ot_p = _Pool(tc, name="outsb", bufs=2)

        vp_all = vplp_p.tile([P, st_n, d], bf16, tag='vp_all')
        lp_all = vplp_p.tile([P, st_n, d], bf16, tag='lp_all')
        pT_all = slab48.tile([P, 2 * dt_n, s], bf16, tag='slab48',
                             name='pT_all')
        vpT_all = pT_all[:, :dt_n, :]
        lpT_all = pT_all[:, dt_n:, :]

        def layernorm_relu(ps, dst, g_nm, be_nm, tagsfx):
            """LN over the free dim of psum tile ps, then relu into dst."""
            stats = sp.tile([P, bn_k, 6], f32, tag="stats" + tagsfx)
            for i in range(bn_k):
                nc.vector.bn_stats(out=stats[:, i, :],
                                   in_=ps[:, i * bn_sub:(i + 1) * bn_sub])
            mv = sp.tile([P, 2], f32, tag="mv" + tagsfx)
            nc.vector.bn_aggr(out=mv[:], in_=stats[:])
            rstd = sp.tile([P, 1], f32, tag="rstd" + tagsfx)
            nc.scalar.activation(out=rstd[:], in_=mv[:, 1:2],
                                 func=AF.Sqrt, bias=eps_sb[:])
            nc.vector.reciprocal(out=rstd[:], in_=rstd[:])
            nmr = sp.tile([P, 1], f32, tag="nmr" + tagsfx)
            nc.vector.tensor_tensor(out=nmr[:], in0=mv[:, 0:1],
                                    in1=rstd[:], op=OP.mult)
            nc.vector.tensor_scalar_mul(out=nmr[:], in0=nmr[:], scalar1=-1.0)
            if trivial:
                nc.scalar.activation(out=dst, in_=ps[:], func=AF.Relu,
                                     bias=nmr[:], scale=rstd[:])
            else:
                nrm = sp.tile([P, d], f32, tag="nrm" + tagsfx, bufs=2)
                nc.vector.tensor_scalar(out=nrm[:], in0=ps[:],
                                        scalar1=mv[:, 0:1], scalar2=rstd[:],
                                        op0=OP.subtract, op1=OP.mult)
                nc.vector.tensor_mul(out=nrm[:], in0=nrm[:], in1=aff[g_nm][:])
                nc.vector.tensor_add(out=nrm[:], in0=nrm[:], in1=aff[be_nm][:])
                nc.scalar.activation(out=dst, in_=nrm[:], func=AF.Relu)

        def linear_into_psum(ps, x_sb, W_sb, bias_nm, n_k):
            for c0, cl in d_chunks:
                for kt in range(n_k):
                    nc.tensor.matmul(
                        ps[:, c0:c0 + cl],
                        x_sb(kt),
                        W_sb[:, kt, c0:c0 + cl],
                        start=(kt == 0),
                        stop=(kt == n_k - 1 and trivial),
                    )
                if not trivial:
                    nc.tensor.matmul(
                        ps[:, c0:c0 + cl], ones_sb[:1, :],
                        b_sb[bias_nm][:1, c0:c0 + cl],
                        start=False, stop=True)

        # ---------- phase A/B: projections ----------
        # packed v/l input, prefetched once per-k-tile into the slab64 slot
        vlT_sb = slab64.tile([P, 2 * dt_n, s], bf16, tag="slab64",
                             name="vlT_sb")
        for j in range(2 * dt_n):
            nc.sync.dma_start(out=vlT_sb[:, j, :],
                              in_=vlT_d.ap()[j * P:(j + 1) * P, :])

        def proj(base, W_d, xp_all, xpT_all, bias_nm, g_nm, be_nm):
            ps_p = _Pool(tc, name="psab" + bias_nm, bufs=3,
                         space=bass.MemorySpace.PSUM)
            W_sb = w_p.tile([P, dt_n, d], bf16, tag="wslab", name="W_sb")
            for j in range(dt_n):
                nc.gpsimd.dma_start(out=W_sb[:, j, :],
                                    in_=W_d.ap()[j * P:(j + 1) * P, :])
            for st in range(st_n):
                ps = ps_p.tile([P, d], f32, tag="ps")
                linear_into_psum(
                    ps, lambda kt: vlT_sb[:, base + kt, st * P:(st + 1) * P],
                    W_sb, bias_nm, dt_n)
                layernorm_relu(ps, xp_all[:, st, :], g_nm, be_nm, "p")
                eng = nc.sync if st % 2 == 0 else nc.scalar
                eng.dma_start_transpose(
                    out=xpT_all[:, :, st * P:(st + 1) * P],
                    in_=xp_all[:, st, :])
            ps_p.close()

        proj(0, vW_d, vp_all, vpT_all, "vb", "vg", "vbeta")
        proj(dt_n, lW_d, lp_all, lpT_all, "lb", "lg", "lbeta")

        # ---------- phase C: sim matmul + exp (+ row sums) ----------
        E_all = slab64.tile([P, st_n, s], bf16, tag="slab64", name="E_all")
        sim_p = _Pool(tc, name="pssim", bufs=8, space=bass.MemorySpace.PSUM)
        for st in range(st_n):
            for t0 in range(tc_n):
                ps = sim_p.tile([P, ch], f32, tag="ps")
                for dt in range(dt_n):
                    nc.tensor.matmul(
                        ps[:],
                        vpT_all[:, dt, st * P:(st + 1) * P],
                        lpT_all[:, dt, t0 * ch:(t0 + 1) * ch],
                        start=(dt == 0), stop=(dt == dt_n - 1))
                nc.scalar.activation(
                    out=E_all[:, st, t0 * ch:(t0 + 1) * ch],
                    in_=ps[:], func=AF.Exp, scale=float(c_scale),
                    accum_out=racc[:, st, t0:t0 + 1])
            rs = sp.tile([P, 1], f32, tag="rs")
            nc.vector.tensor_reduce(out=rs[:], in_=racc[:, st, :],
                                    axis=AX.X, op=OP.add)
            nc.vector.reciprocal(out=rinv_all[:, st:st + 1], in_=rs[:])
        sim_p.close()

        # softmax normalizers: rinv (per row s) to DRAM in s-order for
        # broadcast reload along the free axis
        nc.gpsimd.dma_start(
            out=bass.AP(tensor=rscr_d.ap().tensor, offset=0,
                        ap=[[1, P], [P, st_n]]),
            in_=rinv_all[:])
        # fold 1/D into lp rows (for A^T lp)
        for st in range(st_n):
            nc.vector.tensor_scalar_mul(
                out=lp_all[:, st, :], in0=lp_all[:, st, :],
                scalar1=rinv_all[:, st:st + 1])

        # ---------- phase D: av^T = vp^T E^T (cols scaled by rinv) ----------
        combT = slab48.tile([P, kt_n, s], bf16, tag='slab48', name='combT')
        g2 = 256                      # half-group: 2 s-tiles per E^T stage
        g2_sz = g2 // P
        av_p = _Pool(tc, name="psav", bufs=4, space=bass.MemorySpace.PSUM)
        for g in range(s // g2):
            sts = range(g * g2_sz, (g + 1) * g2_sz)
            at = at_p.tile([P, st_n, g2], bf16, tag="atg")
            for i, st in enumerate(sts):
                eng = nc.sync if st % 2 == 0 else nc.scalar
                eng.dma_start_transpose(
                    out=at[:, :, i * P:(i + 1) * P],
                    in_=E_all[:, st, :])
            rbt = rb_p.tile([P, g2], f32, tag="rbt")
            nc.gpsimd.dma_start(
                out=rbt[:],
                in_=bass.AP(tensor=rscr_d.ap().tensor, offset=g * g2,
                            ap=[[0, P], [1, g2]]))
            for dt in range(dt_n):
                ps = av_p.tile([P, g2], f32, tag="ps")
                for tt in range(st_n):
                    nc.tensor.matmul(
                        ps[:],
                        vp_all[:, tt, dt * P:(dt + 1) * P],
                        at[:, tt, :],
                        start=(tt == 0), stop=(tt == st_n - 1))
                # psum -> sbuf with softmax 1/D applied along the free axis
                nc.vector.tensor_mul(
                    out=combT[:, dt, g * g2:(g + 1) * g2],
                    in0=ps[:], in1=rbt[:])
        av_p.close()

        # ---------- phase E: al^T = (lp')^T E ; phase F: output projection --
        al_p = _Pool(tc, name="psal", bufs=3, space=bass.MemorySpace.PSUM)
        out_p = _Pool(tc, name="psout", bufs=2, space=bass.MemorySpace.PSUM)
        oW_sb = w_p.tile([P, kt_n, d], bf16, tag="wslab", name="oW_sb")
        for j in range(kt_n):
            nc.gpsimd.dma_start(out=oW_sb[:, j, :],
                                in_=oW_d.ap()[j * P:(j + 1) * P, :])

        for t0 in range(tc_n):
            for dt in range(dt_n):
                ps = al_p.tile([P, ch], f32, tag="ps")
                for st in range(st_n):
                    nc.tensor.matmul(
                        ps[:],
                        lp_all[:, st, dt * P:(dt + 1) * P],
                        E_all[:, st, t0 * ch:(t0 + 1) * ch],
                        start=(st == 0), stop=(st == st_n - 1))
                nc.scalar.copy(
                    out=combT[:, dt_n + dt, t0 * ch:(t0 + 1) * ch], in_=ps[:])
            for rt in range(t0 * g_sz, (t0 + 1) * g_sz):
                ps = out_p.tile([P, d], f32, tag="ps")
                linear_into_psum(
                    ps, lambda kt: combT[:, kt, rt * P:(rt + 1) * P],
                    oW_sb, "ob", kt_n)
                ot = ot_p.tile([P, d], f32, tag="ot")
                layernorm_relu(ps, ot[:], "og", "obeta", "o")
                nc.sync.dma_start(out=out_d.ap()[rt * P:(rt + 1) * P, :],
                                  in_=ot[:])
        out_p.close()
        al_p.close()
        ot_p.close()
        rb_p.close()
        at_p.close()
        w_p.close()
        vplp_p.close()
        slab48.close()
        slab64.close()
        sp.close()
        pp.close()

    nc.compile()
    return nc


def _get_program(c_scale: float, trivial: bool, s: int = S, d: int = D):
    key = (round(float(c_scale), 12), trivial, s, d)
    if key not in _BUILD_CACHE:
        _BUILD_CACHE[key] = _build(c_scale, trivial, s, d)
    return _BUILD_CACHE[key]


def _prep_in_maps(vision, language, vW, lW, oW, c_scale, trivial, extras):
    n_b = vision.shape[0]
    vWb = np.ascontiguousarray(np.asarray(vW).astype(BF16))
    lWb = np.ascontiguousarray(np.asarray(lW).astype(BF16))
    oWb = np.ascontiguousarray(np.asarray(oW).astype(BF16))
    in_maps = []
    for b in range(n_b):
        vlT = np.concatenate([vision[b].T, language[b].T], 0)
        m = {
            "vlT": np.ascontiguousarray(vlT.astype(BF16)),
            "vW": vWb, "lW": lWb, "oW": oWb,
        }
        if not trivial:
            m.update(extras)
        in_maps.append(m)
    return in_maps


def kernel(vision_features, language_features, vW, vb, vg, vbeta,
           lW, lb, lg, lbeta, claw, oW, ob, og, obeta):
    from concourse import bass_utils

    vision = np.asarray(vision_features, np.float32)
    language = np.asarray(language_features, np.float32)
    c_scale = float(np.asarray(claw, np.float32).mean()) / TEMPERATURE
    # softmax is computed without max-subtraction: guard that exp can't
    # overflow (|logit| <= |c| * max|sim|; rows have L2 norm <~ sqrt(D))
    assert abs(c_scale) * 1.5 * D < 80.0, "logit scale too large for exp"

    trivial = (
        np.all(np.asarray(vb) == 0) and np.all(np.asarray(lb) == 0)
        and np.all(np.asarray(ob) == 0)
        and np.all(np.asarray(vg) == 1) and np.all(np.asarray(vbeta) == 0)
        and np.all(np.asarray(lg) == 1) and np.all(np.asarray(lbeta) == 0)
        and np.all(np.asarray(og) == 1) and np.all(np.asarray(obeta) == 0)
    )
    extras = {}
    if not trivial:
        extras = {
            "vb": np.asarray(vb, np.float32).reshape(1, D).astype(BF16),
            "lb": np.asarray(lb, np.float32).reshape(1, D).astype(BF16),
            "ob": np.asarray(ob, np.float32).reshape(1, D).astype(BF16),
            "vg": np.asarray(vg, np.float32).reshape(1, D),
            "vbeta": np.asarray(vbeta, np.float32).reshape(1, D),
            "lg": np.asarray(lg, np.float32).reshape(1, D),
            "lbeta": np.asarray(lbeta, np.float32).reshape(1, D),
            "og": np.asarray(og, np.float32).reshape(1, D),
            "obeta": np.asarray(obeta, np.float32).reshape(1, D),
        }

    nc = _get_program(c_scale, trivial)
    in_maps = _prep_in_maps(vision, language, vW, lW, oW,
                            c_scale, trivial, extras)
    res = bass_utils.run_bass_kernel_spmd(nc, in_maps,
                                          core_ids=list(range(B)))
    return np.stack([res.results[b]["out"] for b in range(B)], axis=0)



# revision 2
# speedup vs baseline: 106.5808x; 106.5808x over previous
"""Trainium2 Bass kernel for nn_ClawMatrix (cross-modal claw-matrix alignment).

reference computation per batch element b:
    vp = relu(LN(v @ vW + vb) * vg + vbeta)          [S, D]
    lp = relu(LN(l @ lW + lb) * lg + lbeta)          [S, D]
    sim = vp @ lp.T * (mean(claw) / 0.07)            [S, S]
    A   = softmax(sim, axis=-1)
    av  = A @ vp                                     [S, D]
    al  = A.T @ lp                                   [S, D]
    out = relu(LN([av, al] @ oW + ob) * og + obeta)  [S, D]

Sharding: data-parallel over batch B=8 across the 8 NeuronCores (one batch
element per core, weights replicated, no collectives).

Device-side strategy (per core):
  - all matmuls in bf16 (fp32 PSUM accumulation)
  - LN+ReLU fused into one ScalarE activation (scale=rstd, bias=-mean*rstd)
  - exp fused with row-sum via activation accum_out; softmax normalizer 1/D
    folded into lp (for A^T lp) and into a post-scale of av^T (for A vp),
    so the unnormalized E=exp(c*sim) matrix is used directly in both bmms
  - transposed layouts (vpT, lpT, E^T blocks) produced by DMA XBAR transpose
  - combined^T assembled directly in [2D, S] layout so the output projection
    needs no additional transposes
  - SBUF pressure handled with explicitly managed (non-LIFO) pool lifetimes
"""

import os
import sys
import math
import numpy as np

for _p in ("/opt/trn_rl_repo", "/root/.axon_site/_ro/trn_rl_repo"):
    if os.path.isdir(_p) and _p not in sys.path:
        sys.path.insert(0, _p)

import ml_dtypes  # noqa: E402

BF16 = ml_dtypes.bfloat16

P = 128           # partitions
B = 8             # batch / cores
S = 2048          # sequence
D = 768           # feature dim
EPS = 1e-5
TEMPERATURE = 0.07

_BUILD_CACHE = {}


class _Pool:
    """Manually managed tile-pool lifetime (enter now, exit at any point)."""

    def __init__(self, tc, **kw):
        self._cm = tc.tile_pool(**kw)
        self.pool = self._cm.__enter__()
        self._open = True

    def tile(self, *a, **kw):
        if "name" not in kw:
            kw["name"] = kw.get("tag") or f"t{id(self) % 9973}"
        return self.pool.tile(*a, **kw)

    def close(self):
        if self._open:
            self._cm.__exit__(None, None, None)
            self._open = False


def _build(c_scale: float, trivial: bool, s: int = S, d: int = D):
    """Builds the single-core Bass program. Returns the compiled Bacc module."""
    import concourse.bass as bass
    import concourse.tile as tile
    from concourse import bacc, mybir

    f32 = mybir.dt.float32
    bf16 = mybir.dt.bfloat16
    AF = mybir.ActivationFunctionType
    AX = mybir.AxisListType
    OP = mybir.AluOpType

    st_n = s // P          # number of 128-row tiles over S
    dt_n = d // P          # number of 128-row tiles over D
    kt_n = 2 * dt_n        # k tiles over 2D for the output projection
    ch = 512               # matmul free-dim chunk (one PSUM bank of fp32)
    tc_n = s // ch         # chunks over S
    g_sz = ch // P         # s-tiles per 512-column group
    bn_sub = math.gcd(512, d)
    bn_k = d // bn_sub
    d_chunks = [(i, min(ch, d - i)) for i in range(0, d, ch)]

    nc = bacc.Bacc(
        "TRN2",
        target_bir_lowering=False,
        debug=False,
        enable_asserts=False,
        num_devices=B,
    )

    vlT_d = nc.dram_tensor("vlT", [2 * d, s], bf16, kind="ExternalInput")
    vW_d = nc.dram_tensor("vW", [d, d], bf16, kind="ExternalInput")
    lW_d = nc.dram_tensor("lW", [d, d], bf16, kind="ExternalInput")
    oW_d = nc.dram_tensor("oW", [2 * d, d], bf16, kind="ExternalInput")
    if not trivial:
        vb_d = nc.dram_tensor("vb", [1, d], bf16, kind="ExternalInput")
        lb_d = nc.dram_tensor("lb", [1, d], bf16, kind="ExternalInput")
        ob_d = nc.dram_tensor("ob", [1, d], bf16, kind="ExternalInput")
        vg_d = nc.dram_tensor("vg", [1, d], f32, kind="ExternalInput")
        vbe_d = nc.dram_tensor("vbeta", [1, d], f32, kind="ExternalInput")
        lg_d = nc.dram_tensor("lg", [1, d], f32, kind="ExternalInput")
        lbe_d = nc.dram_tensor("lbeta", [1, d], f32, kind="ExternalInput")
        og_d = nc.dram_tensor("og", [1, d], f32, kind="ExternalInput")
        obe_d = nc.dram_tensor("obeta", [1, d], f32, kind="ExternalInput")
    out_d = nc.dram_tensor("out", [s, d], f32, kind="ExternalOutput")
    rscr_d = nc.dram_tensor("rinv_scratch", [st_n, P], f32, kind="Internal")

    with tile.TileContext(nc) as tc:
        pp = _Pool(tc, name="persist", bufs=1)
        sp = _Pool(tc, name="small", bufs=4)

        eps_sb = pp.tile([P, 1], f32, tag='eps')
        nc.vector.memset(eps_sb[:], EPS)
        racc = pp.tile([P, st_n, tc_n], f32, tag='racc')
        rinv_all = pp.tile([P, st_n], f32, tag='rinv_all')

        if not trivial:
            ones_sb = pp.tile([1, P], bf16, tag="ones_sb")
            nc.vector.memset(ones_sb[:], 1.0)
            b_sb = {}
            aff = {}
            for nm, dd in (("vb", vb_d), ("lb", lb_d), ("ob", ob_d)):
                t = pp.tile([1, d], bf16, tag=nm)
                nc.sync.dma_start(out=t[:], in_=dd.ap())
                b_sb[nm] = t
            for nm, dd in (("vg", vg_d), ("vbeta", vbe_d), ("lg", lg_d),
                           ("lbeta", lbe_d), ("og", og_d), ("obeta", obe_d)):
                t = pp.tile([P, d], f32, tag=nm)
                src = bass.AP(tensor=dd.ap().tensor, offset=0,
                              ap=[[0, P], [1, d]])
                nc.sync.dma_start(out=t[:], in_=src)
                aff[nm] = t

        # Address reuse via same-tag slot cycling (bufs=1):
        #   slab64: vT -> lT -> E_all          (64 KB/part)
        #   slab48: (vpT | lpT) -> combT       (48 KB/part)
        slab64 = _Pool(tc, name="slab64", bufs=1)
        slab48 = _Pool(tc, name="slab48", bufs=1)
        vplp_p = _Pool(tc, name="vplp", bufs=1)
        w_p = _Pool(tc, name="wslab", bufs=1)
        at_p = _Pool(tc, name="at", bufs=2)
        rb_p = _Pool(tc, name="rb", bufs=2)
        ot_p = _Pool(tc, name="outsb", bufs=2)

        vp_all = vplp_p.tile([P, st_n, d], bf16, tag='vp_all')
        lp_all = vplp_p.tile([P, st_n, d], bf16, tag='lp_all')
        pT_all = slab48.tile([P, 2 * dt_n, s], bf16, tag='slab48',
                             name='pT_all')
        vpT_all = pT_all[:, :dt_n, :]
        lpT_all = pT_all[:, dt_n:, :]

        def layernorm_relu(ps, dst, g_nm, be_nm, tagsfx):
            """LN over the free dim of psum tile ps, then relu into dst."""
            stats = sp.tile([P, bn_k, 6], f32, tag="stats" + tagsfx)
            for i in range(bn_k):
                nc.vector.bn_stats(out=stats[:, i, :],
                                   in_=ps[:, i * bn_sub:(i + 1) * bn_sub])
            mv = sp.tile([P, 2], f32, tag="mv" + tagsfx)
            nc.vector.bn_aggr(out=mv[:], in_=stats[:])
            rstd = sp.tile([P, 1], f32, tag="rstd" + tagsfx)
            nc.scalar.activation(out=rstd[:], in_=mv[:, 1:2],
                                 func=AF.Sqrt, bias=eps_sb[:])
            nc.vector.reciprocal(out=rstd[:], in_=rstd[:])
            nmr = sp.tile([P, 1], f32, tag="nmr" + tagsfx)
            nc.vector.tensor_tensor(out=nmr[:], in0=mv[:, 0:1],
                                    in1=rstd[:], op=OP.mult)
            nc.vector.tensor_scalar_mul(out=nmr[:], in0=nmr[:], scalar1=-1.0)
            if trivial:
                nc.scalar.activation(out=dst, in_=ps[:], func=AF.Relu,
                                     bias=nmr[:], scale=rstd[:])
            else:
                nrm = sp.tile([P, d], f32, tag="nrm" + tagsfx, bufs=2)
                nc.vector.tensor_scalar(out=nrm[:], in0=ps[:],
                                        scalar1=mv[:, 0:1], scalar2=rstd[:],
                                        op0=OP.subtract, op1=OP.mult)
                nc.vector.tensor_mul(out=nrm[:], in0=nrm[:], in1=aff[g_nm][:])
                nc.vector.tensor_add(out=nrm[:], in0=nrm[:], in1=aff[be_nm][:])
                nc.scalar.activation(out=dst, in_=nrm[:], func=AF.Relu)

        def linear_into_psum(ps, x_sb, W_sb, bias_nm, n_k):
            for c0, cl in d_chunks:
                for kt in range(n_k):
                    nc.tensor.matmul(
                        ps[:, c0:c0 + cl],
                        x_sb(kt),
                        W_sb[:, kt, c0:c0 + cl],
                        start=(kt == 0),
                        stop=(kt == n_k - 1 and trivial),
                    )
                if not trivial:
                    nc.tensor.matmul(
                        ps[:, c0:c0 + cl], ones_sb[:1, :],
                        b_sb[bias_nm][:1, c0:c0 + cl],
                        start=False, stop=True)

        # ---------- phase A/B: projections ----------
        # packed v/l input, prefetched once per-k-tile into the slab64 slot
        vlT_sb = slab64.tile([P, 2 * dt_n, s], bf16, tag="slab64",
                             name="vlT_sb")
        for j in range(2 * dt_n):
            nc.sync.dma_start(out=vlT_sb[:, j, :],
                              in_=vlT_d.ap()[j * P:(j + 1) * P, :])

        def proj(base, W_d, xp_all, xpT_all, bias_nm, g_nm, be_nm):
            ps_p = _Pool(tc, name="psab" + bias_nm, bufs=3,
                         space=bass.MemorySpace.PSUM)
            W_sb = w_p.tile([P, dt_n, d], bf16, tag="wslab", name="W_sb")
            for j in range(dt_n):
                nc.gpsimd.dma_start(out=W_sb[:, j, :],
                                    in_=W_d.ap()[j * P:(j + 1) * P, :])
            for st in range(st_n):
                ps = ps_p.tile([P, d], f32, tag="ps")
                linear_into_psum(
                    ps, lambda kt: vlT_sb[:, base + kt, st * P:(st + 1) * P],
                    W_sb, bias_nm, dt_n)
                layernorm_relu(ps, xp_all[:, st, :], g_nm, be_nm, "p")
                eng = nc.sync if st % 2 == 0 else nc.scalar
                eng.dma_start_transpose(
                    out=xpT_all[:, :, st * P:(st + 1) * P],
                    in_=xp_all[:, st, :])
            ps_p.close()

        proj(0, vW_d, vp_all, vpT_all, "vb", "vg", "vbeta")
        proj(dt_n, lW_d, lp_all, lpT_all, "lb", "lg", "lbeta")

        # ---------- phase C: sim matmul + exp (+ row sums) ----------
        E_all = slab64.tile([P, st_n, s], bf16, tag="slab64", name="E_all")
        sim_p = _Pool(tc, name="pssim", bufs=8, space=bass.MemorySpace.PSUM)
        for st in range(st_n):
            for t0 in range(tc_n):
                ps = sim_p.tile([P, ch], f32, tag="ps")
                for dt in range(dt_n):
                    nc.tensor.matmul(
                        ps[:],
                        vpT_all[:, dt, st * P:(st + 1) * P],
                        lpT_all[:, dt, t0 * ch:(t0 + 1) * ch],
                        start=(dt == 0), stop=(dt == dt_n - 1))
                nc.scalar.activation(
                    out=E_all[:, st, t0 * ch:(t0 + 1) * ch],
                    in_=ps[:], func=AF.Exp, scale=float(c_scale),
                    accum_out=racc[:, st, t0:t0 + 1])
            rs = sp.tile([P, 1], f32, tag="rs")
            nc.vector.tensor_reduce(out=rs[:], in_=racc[:, st, :],
                                    axis=AX.X, op=OP.add)
            nc.vector.reciprocal(out=rinv_all[:, st:st + 1], in_=rs[:])
        sim_p.close()

        # softmax normalizers: rinv (per row s) to DRAM in s-order for
        # broadcast reload along the free axis
        nc.gpsimd.dma_start(
            out=bass.AP(tensor=rscr_d.ap().tensor, offset=0,
                        ap=[[1, P], [P, st_n]]),
            in_=rinv_all[:])
        # fold 1/D into lp rows (for A^T lp)
        for st in range(st_n):
            nc.vector.tensor_scalar_mul(
                out=lp_all[:, st, :], in0=lp_all[:, st, :],
                scalar1=rinv_all[:, st:st + 1])

        # ---------- phase D: av^T = vp^T E^T (cols scaled by rinv) ----------
        combT = slab48.tile([P, kt_n, s], bf16, tag='slab48', name='combT')
        g2 = 256                      # half-group: 2 s-tiles per E^T stage
        g2_sz = g2 // P
        av_p = _Pool(tc, name="psav", bufs=4, space=bass.MemorySpace.PSUM)
        for g in range(s // g2):
            sts = range(g * g2_sz, (g + 1) * g2_sz)
            at = at_p.tile([P, st_n, g2], bf16, tag="atg")
            for i, st in enumerate(sts):
                eng = nc.sync if st % 2 == 0 else nc.scalar
                eng.dma_start_transpose(
                    out=at[:, :, i * P:(i + 1) * P],
                    in_=E_all[:, st, :])
            rbt = rb_p.tile([P, g2], f32, tag="rbt")
            nc.gpsimd.dma_start(
                out=rbt[:],
                in_=bass.AP(tensor=rscr_d.ap().tensor, offset=g * g2,
                            ap=[[0, P], [1, g2]]))
            for dt in range(dt_n):
                ps = av_p.tile([P, g2], f32, tag="ps")
                for tt in range(st_n):
                    nc.tensor.matmul(
                        ps[:],
                        vp_all[:, tt, dt * P:(dt + 1) * P],
                        at[:, tt, :],
                        start=(tt == 0), stop=(tt == st_n - 1))
                # psum -> sbuf with softmax 1/D applied along the free axis
                nc.vector.tensor_mul(
                    out=combT[:, dt, g * g2:(g + 1) * g2],
                    in0=ps[:], in1=rbt[:])
        av_p.close()

        # ---------- phase E: al^T = (lp')^T E ; phase F: output projection --
        al_p = _Pool(tc, name="psal", bufs=3, space=bass.MemorySpace.PSUM)
        out_p = _Pool(tc, name="psout", bufs=2, space=bass.MemorySpace.PSUM)
        oW_sb = w_p.tile([P, kt_n, d], bf16, tag="wslab", name="oW_sb")
        for j in range(kt_n):
            nc.gpsimd.dma_start(out=oW_sb[:, j, :],
                                in_=oW_d.ap()[j * P:(j + 1) * P, :])

        for t0 in range(tc_n):
            for dt in range(dt_n):
                ps = al_p.tile([P, ch], f32, tag="ps")
                for st in range(st_n):
                    nc.tensor.matmul(
                        ps[:],
                        lp_all[:, st, dt * P:(dt + 1) * P],
                        E_all[:, st, t0 * ch:(t0 + 1) * ch],
                        start=(st == 0), stop=(st == st_n - 1))
                nc.scalar.copy(
                    out=combT[:, dt_n + dt, t0 * ch:(t0 + 1) * ch], in_=ps[:])
            for rt in range(t0 * g_sz, (t0 + 1) * g_sz):
                ps = out_p.tile([P, d], f32, tag="ps")
                linear_into_psum(
                    ps, lambda kt: combT[:, kt, rt * P:(rt + 1) * P],
                    oW_sb, "ob", kt_n)
                ot = ot_p.tile([P, d], f32, tag="ot")
                layernorm_relu(ps, ot[:], "og", "obeta", "o")
                nc.sync.dma_start(out=out_d.ap()[rt * P:(rt + 1) * P, :],
                                  in_=ot[:])
        out_p.close()
        al_p.close()
        ot_p.close()
        rb_p.close()
        at_p.close()
        w_p.close()
        vplp_p.close()
        slab48.close()
        slab64.close()
        sp.close()
        pp.close()

    nc.compile()
    return nc


def _get_program(c_scale: float, trivial: bool, s: int = S, d: int = D):
    key = (round(float(c_scale), 12), trivial, s, d)
    if key not in _BUILD_CACHE:
        _BUILD_CACHE[key] = _build(c_scale, trivial, s, d)
    return _BUILD_CACHE[key]


def _prep_in_maps(vision, language, vW, lW, oW, c_scale, trivial, extras):
    n_b = vision.shape[0]
    vWb = np.ascontiguousarray(np.asarray(vW).astype(BF16))
    lWb = np.ascontiguousarray(np.asarray(lW).astype(BF16))
    oWb = np.ascontiguousarray(np.asarray(oW).astype(BF16))
    in_maps = []
    for b in range(n_b):
        vlT = np.concatenate([vision[b].T, language[b].T], 0)
        m = {
            "vlT": np.ascontiguousarray(vlT.astype(BF16)),
            "vW": vWb, "lW": lWb, "oW": oWb,
        }
        if not trivial:
            m.update(extras)
        in_maps.append(m)
    return in_maps


def _program_and_inmaps(inputs):
    """(compiled program, per-core input maps) for the given full inputs."""
    vision = np.asarray(inputs["vision_features"], np.float32)
    language = np.asarray(inputs["language_features"], np.float32)
    c_scale = float(np.asarray(inputs["claw"], np.float32).mean()) / TEMPERATURE
    nc = _get_program(c_scale, True)
    in_maps = _prep_in_maps(vision, language, inputs["vW"], inputs["lW"],
                            inputs["oW"], c_scale, True, {})
    return nc, in_maps


def kernel(vision_features, language_features, vW, vb, vg, vbeta,
           lW, lb, lg, lbeta, claw, oW, ob, og, obeta):
    from concourse import bass_utils

    vision = np.asarray(vision_features, np.float32)
    language = np.asarray(language_features, np.float32)
    c_scale = float(np.asarray(claw, np.float32).mean()) / TEMPERATURE
    # softmax is computed without max-subtraction: guard that exp can't
    # overflow (|logit| <= |c| * max|sim|; rows have L2 norm <~ sqrt(D))
    assert abs(c_scale) * 1.5 * D < 80.0, "logit scale too large for exp"

    trivial = (
        np.all(np.asarray(vb) == 0) and np.all(np.asarray(lb) == 0)
        and np.all(np.asarray(ob) == 0)
        and np.all(np.asarray(vg) == 1) and np.all(np.asarray(vbeta) == 0)
        and np.all(np.asarray(lg) == 1) and np.all(np.asarray(lbeta) == 0)
        and np.all(np.asarray(og) == 1) and np.all(np.asarray(obeta) == 0)
    )
    extras = {}
    if not trivial:
        extras = {
            "vb": np.asarray(vb, np.float32).reshape(1, D).astype(BF16),
            "lb": np.asarray(lb, np.float32).reshape(1, D).astype(BF16),
            "ob": np.asarray(ob, np.float32).reshape(1, D).astype(BF16),
            "vg": np.asarray(vg, np.float32).reshape(1, D),
            "vbeta": np.asarray(vbeta, np.float32).reshape(1, D),
            "lg": np.asarray(lg, np.float32).reshape(1, D),
            "lbeta": np.asarray(lbeta, np.float32).reshape(1, D),
            "og": np.asarray(og, np.float32).reshape(1, D),
            "obeta": np.asarray(obeta, np.float32).reshape(1, D),
        }

    nc = _get_program(c_scale, trivial)
    in_maps = _prep_in_maps(vision, language, vW, lW, oW,
                            c_scale, trivial, extras)
    res = bass_utils.run_bass_kernel_spmd(nc, in_maps,
                                          core_ids=list(range(B)))
    return np.stack([res.results[b]["out"] for b in range(B)], axis=0)

